# revision 30
# baseline (speedup 1.0000x reference)
"""Multi-head attention (B=2, S=2048, D=1024, H=16) on 8 Trainium2 cores.

Sharding: core c handles batch b = c//4 and head group g = c%4 (4 heads,
256 of the 1024 QKV output columns). Each core:

  1. Projects q/k in transposed layout qT/kT [dh, s] (lhsT = W.T column
     slice, rhs = x.T), v in natural layout [s, dh] (lhsT = x.T tile,
     rhs = W.T slice). q/k biases are folded into the PSUM eviction as a
     per-partition tensor_scalar_add on DVE; the v bias is a rank-1
     matmul accumulation (ones ⊗ bias) into the same PSUM group.
  2. Attention per head in transposed layout: logitsT[sk, sq] tile =
     kT_tile.T @ qT (single K=64 matmul), expw = Exp(scale*logits) on ACT
     (scale = 1/sqrt(D) folded into the activation's free affine),
     masked by multiplying with keepT = (~mask).T in bf16 {0,1} — exact,
     since exp(-1e9) underflows to 0 in fp32 so zeroing exp entries is
     identical to the reference's additive -1e9 mask.
  3. PV with a ones-augmented V: out_augT[dh+1, sq] += [v|1].T @ expw —
     row 64 accumulates the softmax denominator for free.
  4. PE-transposes out_augT back to natural [s, dh] in 128-col blocks,
     normalizes rows by 1/rowsum (per-partition scalar), DMAs out.

Matmuls run in bf16 (inputs cast on host), accumulation in fp32 PSUM.
"""

import numpy as np

B, S, D, H = 2, 2048, 1024, 16
HD = D // H  # 64
HEADS_PER_CORE = 4
COLS = HEADS_PER_CORE * HD  # 256
N_CORES = 8
KT = D // 128  # 8 contraction tiles for projections
ST = S // 128  # 16 s tiles
SCALE = 1.0 / np.sqrt(np.float32(D))

_cache = {}


def _build_nc():
    import concourse.bass as bass
    import concourse.mybir as mybir
    import concourse.tile as tile
    from concourse.masks import make_identity

    f32 = mybir.dt.float32
    bf16 = mybir.dt.bfloat16

    nc = bass.Bass(trn_type="TRN2")

    xT = nc.dram_tensor("xT", [D, S], bf16, kind="ExternalInput")
    wq = nc.dram_tensor("wq", [D, COLS], bf16, kind="ExternalInput")
    wk = nc.dram_tensor("wk", [D, COLS], bf16, kind="ExternalInput")
    wv = nc.dram_tensor("wv", [D, COLS], bf16, kind="ExternalInput")
    bq = nc.dram_tensor("bq", [128, 2], f32, kind="ExternalInput")
    bk = nc.dram_tensor("bk", [128, 2], f32, kind="ExternalInput")
    bv = nc.dram_tensor("bv", [1, COLS], bf16, kind="ExternalInput")
    keepT = nc.dram_tensor("keepT", [S, S], bf16, kind="ExternalInput")
    o = nc.dram_tensor("o", [S, COLS], f32, kind="ExternalOutput")

    with tile.TileContext(nc) as tc:
        with (
            tc.tile_pool(name="singles", bufs=1) as singles,
            tc.tile_pool(name="persist", bufs=1) as persist,
            tc.tile_pool(name="big_ps", bufs=2, space="PSUM") as big_ps,
            tc.tile_pool(name="pv_ps", bufs=2, space="PSUM") as pv_ps,
            tc.tile_pool(name="tr_ps", bufs=2, space="PSUM") as tr_ps,
            tc.tile_pool(name="expw", bufs=4) as expw_pool,
            tc.tile_pool(name="expw2", bufs=4) as expw2_pool,
            tc.tile_pool(name="tails", bufs=4) as tails,
        ):
            # ---- constants ----
            ones_row = singles.tile([1, 512], bf16)
            nc.vector.memset(ones_row, 1.0)
            ones_col = singles.tile([1, 128], bf16)
            nc.vector.memset(ones_col, 1.0)
            identity = singles.tile([128, 128], f32)
            make_identity(nc, identity)
            bq_sb = singles.tile([128, 2], f32)
            nc.sync.dma_start(out=bq_sb, in_=bq[:, :])
            bk_sb = singles.tile([128, 2], f32)
            nc.sync.dma_start(out=bk_sb, in_=bk[:, :])
            bv_sb = singles.tile([1, COLS], bf16)
            nc.sync.dma_start(out=bv_sb, in_=bv[:, :])

            # ---- bulk inputs ----
            wq_sb = persist.tile([128, KT, COLS], bf16)
            wk_sb = persist.tile([128, KT, COLS], bf16)
            wv_sb = persist.tile([128, KT, COLS], bf16)
            for w_sb, w_dram in ((wk_sb, wk), (wq_sb, wq), (wv_sb, wv)):
                nc.sync.dma_start(
                    out=w_sb,
                    in_=w_dram[:, :].rearrange("(kt p) c -> p kt c", p=128),
                )
            xT_sb = persist.tile([128, KT, S], bf16)
            xT_r = xT[:, :].rearrange("(kt p) s -> p kt s", p=128)
            for c in range(4):
                nc.sync.dma_start(
                    out=xT_sb[:, 2 * c : 2 * c + 2, :],
                    in_=xT_r[:, 2 * c : 2 * c + 2, :],
                )
            keepT_sb = persist.tile([128, ST, S], bf16)
            keepT_r = keepT[:, :].rearrange("(i p) s -> p i s", p=128)
            nc.sync.dma_start(out=keepT_sb[:, 0:8, :], in_=keepT_r[:, 0:8, :])
            nc.sync.dma_start(out=keepT_sb[:, 8:16, :], in_=keepT_r[:, 8:16, :])

            # ---- QKV projection ----
            # qT/kT: [128 (2 heads of dh), blk, s]; head h lives at
            # partitions (h%2)*64.. of block h//2.
            qT_sb = persist.tile([128, 2, S], bf16)
            kT_sb = persist.tile([128, 2, S], bf16)

            def project_qk_group(which, blk, jh):
                w_sb, b_sb, dst = (
                    (wq_sb, bq_sb, qT_sb),
                    (wk_sb, bk_sb, kT_sb),
                )[which]
                ps = big_ps.tile([128, 1024], f32, tag="big")
                for nn in range(2):
                    sl = ps[:, nn * 512 : (nn + 1) * 512]
                    for kt in range(KT):
                        nc.tensor.matmul(
                            sl,
                            lhsT=w_sb[:, kt, blk * 128 : (blk + 1) * 128],
                            rhs=xT_sb[
                                :, kt, jh * 1024 + nn * 512 : jh * 1024 + (nn + 1) * 512
                            ],
                            start=(kt == 0),
                            stop=(kt == KT - 1),
                            skip_group_check=True,
                        )
                nc.vector.tensor_scalar_add(
                    out=dst[:, blk, jh * 1024 : (jh + 1) * 1024],
                    in0=ps,
                    scalar1=b_sb[:, blk : blk + 1],
                )


            def project_qk_halfgroup(which, blk, jh, nn):
                w_sb, b_sb, dst = (
                    (wq_sb, bq_sb, qT_sb),
                    (wk_sb, bk_sb, kT_sb),
                )[which]
                ps = tr_ps.tile([128, 512], f32, tag="tr")
                for kt in range(KT):
                    nc.tensor.matmul(
                        ps,
                        lhsT=w_sb[:, kt, blk * 128 : (blk + 1) * 128],
                        rhs=xT_sb[
                            :, kt, jh * 1024 + nn * 512 : jh * 1024 + (nn + 1) * 512
                        ],
                        start=(kt == 0),
                        stop=(kt == KT - 1),
                        skip_group_check=True,
                    )
                nc.vector.tensor_scalar_add(
                    out=dst[
                        :, blk, jh * 1024 + nn * 512 : jh * 1024 + (nn + 1) * 512
                    ],
                    in0=ps,
                    scalar1=b_sb[:, blk : blk + 1],
                )

            # v in natural layout, augmented with a ones column per head:
            # v_aug[p, st, h, 0:64] = v, v_aug[p, st, h, 64] = 1
            v_aug = persist.tile([128, ST, HEADS_PER_CORE, HD + 1], bf16)
            nc.vector.memset(v_aug[:, :, :, HD : HD + 1], 1.0)

            def project_v(st):
                psv = tr_ps.tile([128, COLS], f32, tag="tr")
                nc.tensor.matmul(
                    psv,
                    lhsT=ones_col[:, :],
                    rhs=bv_sb[:, :],
                    start=True,
                    stop=False,
                    skip_group_check=True,
                )
                for kt in range(KT):
                    nc.tensor.matmul(
                        psv,
                        lhsT=xT_sb[:, kt, st * 128 : (st + 1) * 128],
                        rhs=wv_sb[:, kt, :],
                        start=False,
                        stop=(kt == KT - 1),
                        skip_group_check=True,
                    )
                nc.vector.tensor_copy(
                    out=v_aug[:, st, :, 0:HD],
                    in_=psv.rearrange("p (h d) -> p h d", h=HEADS_PER_CORE),
                )

            # ---- attention ----
            # Head pair (2hp, 2hp+1) shares one [128,1024] logits PSUM tile:
            # head e in cols 0-511 (PE rows 0-63), head o in cols 512-1023
            # (PE rows 64-127 via auto tile_position). The two K=64 matmuls
            # are adjacent and run concurrently on disjoint row groups, and
            # one wide exp covers both heads.
            def attention_pair(hp, filler=None):
                blk = hp
                it = 0
                for j in range(4):  # sq blocks of 512
                    pvs = [
                        pv_ps.tile([HD + 1, 512], f32, tag="pv", name=f"pv{e}")
                        for e in range(2)
                    ]
                    for i in range(ST):  # sk tiles of 128
                        if filler is not None:
                            filler(it)
                        it += 1
                        lgp = big_ps.tile([128, 1024], f32, tag="big")
                        for e in range(2):
                            po = e * 64
                            nc.tensor.matmul(
                                lgp[:, e * 512 : (e + 1) * 512],
                                lhsT=kT_sb[
                                    po : po + 64, blk, i * 128 : (i + 1) * 128
                                ],
                                rhs=qT_sb[
                                    po : po + 64, blk, j * 512 : (j + 1) * 512
                                ],
                                start=True,
                                stop=True,
                                skip_group_check=True,
                            )
                        ex = expw_pool.tile([128, 1024], bf16)
                        nc.scalar.activation(
                            out=ex,
                            in_=lgp,
                            func=mybir.ActivationFunctionType.Exp,
                            scale=float(SCALE),
                        )
                        # mask: multiply both heads' halves by the same keepT
                        # slice, read twice via a stride-0 broadcast dim
                        ex2 = expw2_pool.tile([128, 1024], bf16)
                        k_ap = keepT_sb[:, i, j * 512 : (j + 1) * 512]
                        k_bcast = bass.AP(
                            tensor=k_ap.tensor,
                            offset=k_ap.offset,
                            ap=[k_ap.ap[0], [0, 2], *k_ap.ap[1:]],
                        )
                        nc.vector.tensor_mul(
                            out=ex2.rearrange("p (e n) -> p e n", e=2),
                            in0=ex.rearrange("p (e n) -> p e n", e=2),
                            in1=k_bcast,
                        )
                        for e in range(2):
                            nc.tensor.matmul(
                                pvs[e],
                                lhsT=v_aug[:, i, 2 * hp + e, :],
                                rhs=ex2[:, e * 512 : (e + 1) * 512],
                                start=(i == 0),
                                stop=(i == ST - 1),
                                skip_group_check=True,
                            )
                    # tail: evict both heads first (frees pv slots for the
                    # next block), then transpose/normalize/store
                    pv_sbs = []
                    for e in range(2):
                        pv_sb = tails.tile(
                            [HD + 1, 512], f32, tag="pvsb", name=f"pv_sb{e}"
                        )
                        nc.vector.tensor_copy(out=pv_sb, in_=pvs[e])
                        pv_sbs.append(pv_sb)
                    for e in range(2):
                        h = 2 * hp + e
                        pv_sb = pv_sbs[e]
                        ob = tails.tile([128, 4, HD], f32, tag="ob")
                        for c in range(4):
                            tr = tr_ps.tile([128, HD + 1], f32, tag="tr")
                            nc.tensor.transpose(
                                out=tr,
                                in_=pv_sb[:, c * 128 : (c + 1) * 128],
                                identity=identity[0 : HD + 1, 0 : HD + 1],
                            )
                            rc = tails.tile([128, 1], f32, tag="rc")
                            nc.vector.reciprocal(out=rc, in_=tr[:, HD : HD + 1])
                            nc.vector.tensor_scalar_mul(
                                out=ob[:, c, :], in0=tr[:, 0:HD], scalar1=rc
                            )
                        nc.sync.dma_start(
                            out=o[
                                j * 512 : (j + 1) * 512, h * HD : (h + 1) * HD
                            ].rearrange("(c p) d -> p c d", p=128),
                            in_=ob,
                        )

            # Emission order = PE program order: k/q block 0 first (so
            # attention can start), then v, then attention on heads 0/1 with
            # qk block-1 projection groups sprinkled into PE slack.
            for jh in range(2):
                project_qk_group(1, 0, jh)  # k blk0
            for jh in range(2):
                project_qk_group(0, 0, jh)  # q blk0
            for st in range(ST):
                project_v(st)
            # qk block 1 rides in attention-phase PE slack (ACT-bound there),
            # via 1-bank tr-pool psums so the logits double-buffer is untouched
            qk1_half = [
                (w, 1, jh, nn) for w in range(2) for jh in range(2) for nn in range(2)
            ]

            def qk1_filler(it):
                if it in (5, 10, 22, 27, 38, 43, 53, 58) and qk1_half:
                    project_qk_halfgroup(*qk1_half.pop(0))

            attention_pair(0, filler=qk1_filler)
            attention_pair(1)

    # Workaround: this container's walrus encodes at most one sync wait per
    # instruction — split multi-wait instructions into single-wait NoOps.
    _split_multiwait(nc)
    return nc


def _split_multiwait(nc, max_waits: int = 1):
    import concourse.mybir as mybir

    for f in nc.m.functions:
        for blk in f.blocks:
            out = []
            changed = False
            for inst in blk.instructions:
                si = inst.sync_info
                if si is not None and len(si.on_wait) > max_waits:
                    waits = list(si.on_wait)
                    extra = waits[: len(waits) - max_waits]
                    keep = waits[len(waits) - max_waits :]
                    for k, w in enumerate(extra):
                        out.append(
                            mybir.InstNoOp(
                                name=f"{inst.name}-wfx{k}",
                                engine=inst.engine,
                                sync_info=mybir.SyncInfo(on_wait=[w], on_update=[]),
                                bass_nofuse=True,
                            )
                        )
                    inst.sync_info = mybir.SyncInfo(
                        on_wait=keep, on_update=list(si.on_update)
                    )
                    changed = True
                out.append(inst)
            if changed:
                blk.instructions = out


def _prep_in_maps(x, mask, Wq, bq, Wk, bk, Wv, bv):
    import ml_dtypes

    bf16 = ml_dtypes.bfloat16
    x = np.asarray(x, np.float32)
    mask = np.asarray(mask, bool)

    xT_b = [np.ascontiguousarray(x[b].T).astype(bf16) for b in range(B)]
    keepT_b = [
        np.ascontiguousarray((~mask[b, 0]).T).astype(bf16) for b in range(B)
    ]
    WqT = np.asarray(Wq, np.float32).T.astype(bf16)
    WkT = np.asarray(Wk, np.float32).T.astype(bf16)
    WvT = np.asarray(Wv, np.float32).T.astype(bf16)
    bq32 = np.asarray(bq, np.float32)
    bk32 = np.asarray(bk, np.float32)
    bv = np.asarray(bv, np.float32).astype(bf16)

    in_maps = []
    for c in range(N_CORES):
        b, g = divmod(c, 4)
        cols = slice(g * COLS, (g + 1) * COLS)
        in_maps.append(
            {
                "xT": xT_b[b],
                "wq": np.ascontiguousarray(WqT[:, cols]),
                "wk": np.ascontiguousarray(WkT[:, cols]),
                "wv": np.ascontiguousarray(WvT[:, cols]),
                "bq": np.ascontiguousarray(bq32[cols].reshape(2, 128).T),
                "bk": np.ascontiguousarray(bk32[cols].reshape(2, 128).T),
                "bv": np.ascontiguousarray(bv[cols].reshape(1, COLS)),
                "keepT": keepT_b[b],
            }
        )
    return in_maps


def kernel(x, mask, Wq, bq, Wk, bk, Wv, bv, _trace=False):
    from concourse.bass_utils import run_bass_kernel_spmd

    if "nc" not in _cache:
        _cache["nc"] = _build_nc()
    nc = _cache["nc"]

    in_maps = _prep_in_maps(x, mask, Wq, bq, Wk, bk, Wv, bv)
    res = run_bass_kernel_spmd(
        nc, in_maps, core_ids=list(range(N_CORES)), trace=_trace
    )
    _cache["last_result"] = res

    out = np.empty((B, S, D), np.float32)
    for c in range(N_CORES):
        b, g = divmod(c, 4)
        out[b, :, g * COLS : (g + 1) * COLS] = res.results[c]["o"]
    return out
